# revision 3
# baseline (speedup 1.0000x reference)
"""MiniMaxText01 linear attention on 8 trn2 NeuronCores — fused bf16 version.

Tensor-parallel over heads (4 heads/core). Single fused pass per core:
for each 512-token chunk: qkv/gate projections (bf16 matmuls, weights
SBUF-resident), blocked lightning attention (BLOCK=256, kv state in SBUF
bf16, per-block decay folded into a diagonal-matrix matmul), gating
(sigmoid via tanh so the ACT engine never switches table sets), and the
out projection. Each core emits:
  pout = (gate * attn * 0.5*norm_w) @ w_out   [4096, 2048]  (0.5*normw
         pre-folded into w_out rows on the host)
  ssq  = sum over this core's channels of attn^2   [1, 4096]
Host: out = sum_c(pout) * rsqrt(sum_c(ssq)/4096 + eps).
"""
import math
import numpy as np
from contextlib import ExitStack

import ml_dtypes
import concourse.bass as bass
import concourse.tile as tile
import concourse.mybir as mybir
from concourse import bacc
from concourse.bass_utils import run_bass_kernel_spmd

FP32 = mybir.dt.float32
BF16 = mybir.dt.bfloat16
AF = mybir.ActivationFunctionType
BF = ml_dtypes.bfloat16

SEQ = 4096
HIDDEN = 2048
NUM_HEADS = 32
HEAD_DIM = 128
INNER = NUM_HEADS * HEAD_DIM
BLOCK = 256
EPS = 1e-5
N_CORES = 8
HPC = NUM_HEADS // N_CORES          # 4 heads per core
IN_PC = HPC * HEAD_DIM              # 512 inner channels per core
P = 128

CH = 512                            # tokens per chunk
NT = SEQ // CH                      # 8 chunks
HC = HIDDEN // P                    # 16 hidden sub-blocks


REGION_LOG = []


def _mark(label, ins):
    try:
        REGION_LOG.append((label, ins.ins.name))
    except Exception:
        pass


def build_nc(repeat: int = 1, timing: bool = False):
    REGION_LOG.clear()
    nc = bacc.Bacc("TRN2", target_bir_lowering=False)

    # In timing mode every big tensor is Internal (device-resident garbage)
    # so per-call host<->device traffic is tiny and the R-repeat wall-clock
    # delta isolates pure HW execution time.
    big = {"kind": "Internal"} if timing else {"kind": "ExternalInput"}
    xt_d = nc.dram_tensor("xtb", [HIDDEN, SEQ], BF16, **big)
    wq_d = nc.dram_tensor("wqb", [HIDDEN, IN_PC], BF16, **big)
    wk_d = nc.dram_tensor("wkb", [HIDDEN, IN_PC], BF16, **big)
    wv_d = nc.dram_tensor("wvb", [HIDDEN, IN_PC], BF16, **big)
    wg_d = nc.dram_tensor("wgb", [HIDDEN, IN_PC], BF16, **big)
    wo_d = nc.dram_tensor("wob", [IN_PC, HIDDEN], BF16, **big)
    qdec_d = nc.dram_tensor("qdec", [HPC, P, BLOCK], BF16, **big)
    dmask0_d = nc.dram_tensor("dmask0", [HPC, P, BLOCK], FP32, **big)
    dmask1_d = nc.dram_tensor("dmask1", [HPC, P, BLOCK], FP32, **big)
    kdec_d = nc.dram_tensor("kdec", [HPC, 2, P, 1], FP32, **big)
    bdiag_d = nc.dram_tensor("bdiag", [P, HPC, P], BF16, **big)
    ident_d = nc.dram_tensor("identb", [P, P], BF16, **big)
    ones_d = nc.dram_tensor("onesb", [P, 1], BF16, kind="ExternalInput")
    kv0_d = nc.dram_tensor("kv0b", [HPC, P, P], BF16, **big)

    if timing:
        pout_d = nc.dram_tensor("pout", [SEQ, HIDDEN], FP32, kind="Internal")
        probe_d = nc.dram_tensor("probe", [P, 4], FP32, kind="ExternalOutput")
    else:
        pout_d = nc.dram_tensor("pout", [SEQ, HIDDEN], FP32, kind="ExternalOutput")
    ssq_d = nc.dram_tensor("ssq", [1, SEQ], FP32, kind="ExternalOutput")

    with tile.TileContext(nc) as tc, ExitStack() as ctx:
        const = ctx.enter_context(tc.tile_pool(name="const", bufs=1))
        wpool = ctx.enter_context(tc.tile_pool(name="w", bufs=1))
        xpool = ctx.enter_context(tc.tile_pool(name="x", bufs=1))
        apool = ctx.enter_context(tc.tile_pool(name="a", bufs=1))
        spool = ctx.enter_context(tc.tile_pool(name="s", bufs=1))
        psum = ctx.enter_context(tc.tile_pool(name="psum", bufs=1, space="PSUM"))

        HH = HC // 2
        xt_r = xt_d[:].rearrange("(hc p) n -> p hc n", p=P)

        # startup order: wq/xh0 interleaved so the first matmul chain can
        # begin ~6us in; wo (needed last) loads last.
        wq_t = wpool.tile([P, HC, IN_PC], BF16)
        wk_t = wpool.tile([P, HC, IN_PC], BF16)
        wv_t = wpool.tile([P, HC, IN_PC], BF16)
        wg_t = wpool.tile([P, HC, IN_PC], BF16)
        wo_t = wpool.tile([P, HPC, HIDDEN], BF16)
        wq_r = wq_d[:].rearrange("(hc p) m -> p hc m", p=P)

        xh0 = xpool.tile([P, HC, CH], BF16, tag="xh", bufs=2)
        for half in range(2):
            nc.sync.dma_start(wq_t[:, half * HH:(half + 1) * HH, :],
                              wq_r[:, half * HH:(half + 1) * HH, :])
            nc.sync.dma_start(xh0[:, half * HH:(half + 1) * HH, :],
                              xt_r[:, half * HH:(half + 1) * HH, 0:CH])
        for w_t, w_d in ((wk_t, wk_d), (wv_t, wv_d), (wg_t, wg_d)):
            w_r = w_d[:].rearrange("(hc p) m -> p hc m", p=P)
            for half in range(2):
                nc.sync.dma_start(w_t[:, half * HH:(half + 1) * HH, :],
                                  w_r[:, half * HH:(half + 1) * HH, :])

        ident_t = const.tile([P, P], BF16)
        nc.sync.dma_start(ident_t[:], ident_d[:])
        qdec_t = const.tile([P, HPC, BLOCK], BF16)
        nc.sync.dma_start(qdec_t[:], qdec_d[:].rearrange("h p i -> p h i"))
        dmask0_t = const.tile([P, HPC, BLOCK], FP32)
        nc.sync.dma_start(dmask0_t[:], dmask0_d[:].rearrange("h p i -> p h i"))
        dmask1_t = const.tile([P, HPC, BLOCK], FP32)
        nc.sync.dma_start(dmask1_t[:], dmask1_d[:].rearrange("h p i -> p h i"))
        kdec_t = const.tile([P, HPC, 2, 1], FP32)
        nc.sync.dma_start(kdec_t[:], kdec_d[:].rearrange("h s p o -> p h s o"))
        bdiag_t = const.tile([P, HPC, P], BF16)
        nc.sync.dma_start(bdiag_t[:], bdiag_d[:])
        ones_t = const.tile([P, 1], BF16)
        nc.sync.dma_start(ones_t[:], ones_d[:])
        nc.sync.dma_start(wo_t[:], wo_d[:].rearrange("(h p) n -> p h n", p=P))
        kv_t = const.tile([P, HPC, P], BF16)

        for _rep in range(repeat):
            nc.sync.dma_start(kv_t[:], kv0_d[:].rearrange("h d e -> d h e"))

            pending = []          # (t, attn_t, gTt, b) awaiting gating/ssq/outproj
            tail_state = {}       # t -> (gA, sq) tiles

            def emit_tail(t, attn_t, gTt, b):
                # gating for half-chunk b (0.5*normw folded into w_out):
                # gA = attn*(1+tanh)
                if t not in tail_state:
                    gA = apool.tile([P, HPC, CH], BF16, tag="gA", bufs=2, name=f"gA{t}")
                    sq = apool.tile([P, HPC, CH], BF16, tag="sq", bufs=2, name=f"sq{t}")
                    tail_state[t] = (gA, sq)
                gA, sq = tail_state[t]
                bsl = slice(b * BLOCK, (b + 1) * BLOCK)
                nc.vector.tensor_mul(gA[:, :, bsl], attn_t[:, :, bsl], gTt[:, :, bsl])
                nc.vector.tensor_add(gA[:, :, bsl], gA[:, :, bsl], attn_t[:, :, bsl])
                nc.vector.tensor_mul(sq[:, :, bsl], attn_t[:, :, bsl], attn_t[:, :, bsl])

                # ssq = ones^T @ sq for this half
                pss = psum.tile([1, BLOCK], FP32, tag="psA", bufs=2)
                for h in range(HPC):
                    mm = nc.tensor.matmul(pss[:], ones_t[:], sq[:, h, bsl],
                                     start=(h == 0), stop=(h == HPC - 1))
                    if h == 0:
                        _mark(f"t{t}.ssq", mm)
                ssb = spool.tile([1, BLOCK], FP32, tag="ssb", bufs=2)
                nc.scalar.copy(ssb[:], pss[:])
                nc.sync.dma_start(
                    ssq_d[:, t * CH + b * BLOCK: t * CH + (b + 1) * BLOCK], ssb[:])

                # out projection for the two m-tiles of this half
                for m in (2 * b, 2 * b + 1):
                    ob = spool.tile([P, HIDDEN], FP32, tag="ob", bufs=2)
                    for nt in range(4):
                        ps = psum.tile([P, 512], FP32, tag="psA", bufs=2)
                        for h in range(HPC):
                            mm = nc.tensor.matmul(
                                ps[:], gA[:, h, m * P:(m + 1) * P],
                                wo_t[:, h, nt * 512:(nt + 1) * 512],
                                start=(h == 0), stop=(h == HPC - 1))
                            if h == 0:
                                _mark(f"t{t}.op.m{m}.n{nt}", mm)
                        nc.vector.tensor_copy(ob[:, nt * 512:(nt + 1) * 512], ps[:])
                        if nt == 1:
                            nc.sync.dma_start(
                                pout_d[t * CH + m * P: t * CH + (m + 1) * P, 0:1024],
                                ob[:, 0:1024])
                    nc.sync.dma_start(
                        pout_d[t * CH + m * P: t * CH + (m + 1) * P, 1024:HIDDEN],
                        ob[:, 1024:HIDDEN])

            for t in range(NT):
                tsl = slice(t * CH, (t + 1) * CH)
                if t == 0:
                    xh = xh0
                else:
                    xh = xpool.tile([P, HC, CH], BF16, tag="xh", bufs=2)
                    for half in range(2):
                        nc.sync.dma_start(
                            xh[:, half * HH:(half + 1) * HH, :],
                            xt_r[:, half * HH:(half + 1) * HH, tsl])

                qT = apool.tile([P, HPC, CH], BF16, tag="qT", bufs=2)
                kT = apool.tile([P, HPC, CH], BF16, tag="kT", bufs=2)
                v_t = apool.tile([P, 4, IN_PC], BF16, tag="v", bufs=2)
                gTt = apool.tile([P, HPC, CH], BF16, tag="g", bufs=2)

                # q/k projections: psum chains over hidden, silu -> bf16
                for wi, (w_t, dst) in enumerate(((wq_t, qT), (wk_t, kT))):
                    for h in range(HPC):
                        ps = psum.tile([P, CH], FP32, tag="psA", bufs=2)
                        for hc in range(HC):
                            mm = nc.tensor.matmul(
                                ps[:], w_t[:, hc, h * P:(h + 1) * P],
                                xh[:, hc, :],
                                start=(hc == 0), stop=(hc == HC - 1))
                            if hc == 0:
                                _mark(f"t{t}.qk{wi}.h{h}.mm", mm)
                        nc.scalar.activation(dst[:, h, :], ps[:], AF.Silu)

                # v projection (natural layout), silu -> bf16
                for t2 in range(4):
                    ps = psum.tile([P, IN_PC], FP32, tag="psA", bufs=2)
                    for hc in range(HC):
                        mm = nc.tensor.matmul(
                            ps[:], xh[:, hc, t2 * P:(t2 + 1) * P],
                            wv_t[:, hc, :],
                            start=(hc == 0), stop=(hc == HC - 1))
                        if hc == 0:
                            _mark(f"t{t}.v.{t2}.mm", mm)
                    nc.scalar.activation(v_t[:, t2, :], ps[:], AF.Silu)

                # gate projection: tanh(x/2) -> bf16 (gate = 0.5*(1+tanh))
                for h in range(HPC):
                    ps = psum.tile([P, CH], FP32, tag="psA", bufs=2)
                    for hc in range(HC):
                        mm = nc.tensor.matmul(
                            ps[:], wg_t[:, hc, h * P:(h + 1) * P],
                            xh[:, hc, :],
                            start=(hc == 0), stop=(hc == HC - 1))
                        if hc == 0:
                            _mark(f"t{t}.g.h{h}.mm", mm)
                    nc.scalar.activation(gTt[:, h, :], ps[:], AF.Tanh, scale=0.5)

                # pipeline: previous chunk's gating/ssq/outproj emitted here so
                # its PE work interleaves with this chunk's attention stalls
                while pending:
                    emit_tail(*pending.pop(0))
                tail_state.pop(t - 2, None)

                attn_t = apool.tile([P, HPC, CH], BF16, tag="attn", bufs=2)

                for b in range(CH // BLOCK):
                    t0 = b * BLOCK
                    for h in range(HPC):
                        hsl = slice(h * P, (h + 1) * P)
                        # scores (transposed): sT[j, i] = k_j . q_i
                        ps01 = psum.tile([P, 2, BLOCK], FP32, tag="ps_s", bufs=1)
                        mm = nc.tensor.matmul(ps01[:, 0, :], kT[:, h, t0:t0 + P],
                                         qT[:, h, t0:t0 + BLOCK],
                                         start=True, stop=True)
                        _mark(f"t{t}.b{b}.h{h}.score", mm)
                        nc.tensor.matmul(ps01[:, 1, :], kT[:, h, t0 + P:t0 + BLOCK],
                                         qT[:, h, t0:t0 + BLOCK],
                                         start=True, stop=True)
                        s0 = spool.tile([P, BLOCK], BF16, tag="s0", bufs=2)
                        nc.vector.tensor_mul(s0[:], ps01[:, 0, :], dmask0_t[:, h, :])
                        s1 = spool.tile([P, BLOCK], BF16, tag="s1", bufs=2)
                        nc.vector.tensor_mul(s1[:], ps01[:, 1, :], dmask1_t[:, h, :])
                        qd = spool.tile([P, BLOCK], BF16, tag="qd", bufs=2)
                        nc.vector.tensor_mul(qd[:], qT[:, h, t0:t0 + BLOCK],
                                             qdec_t[:, h, :])
                        # k natural (transposed back) with k-decay folded in
                        kn = []
                        pst = psum.tile([P, 2, P], BF16, tag="ps_tr", bufs=2)
                        for sub in range(2):
                            mm = nc.tensor.transpose(
                                pst[:, sub, :], kT[:, h, t0 + sub * P:t0 + (sub + 1) * P],
                                ident_t[:])
                            _mark(f"t{t}.b{b}.h{h}.tr{sub}", mm)
                            knt = spool.tile([P, P], BF16, tag=f"kn{sub}", bufs=2)
                            nc.scalar.activation(knt[:], pst[:, sub, :], AF.Copy,
                                                 scale=kdec_t[:, h, sub, :])
                            kn.append(knt)
                        # attention output (transposed): inter + intra
                        pso = psum.tile([P, BLOCK], FP32, tag="ps_o", bufs=2)
                        mm = nc.tensor.matmul(pso[:], kv_t[:, h, :], qd[:],
                                         start=True, stop=False)
                        _mark(f"t{t}.b{b}.h{h}.pso", mm)
                        nc.tensor.matmul(pso[:], v_t[:, 2 * b, hsl], s0[:],
                                         start=False, stop=False)
                        nc.tensor.matmul(pso[:], v_t[:, 2 * b + 1, hsl], s1[:],
                                         start=False, stop=True)
                        nc.vector.tensor_copy(attn_t[:, h, t0:t0 + BLOCK], pso[:])
                        # kv update in psum: psk = bd*kv + (k*kdec)^T v
                        psk = psum.tile([P, P], FP32, tag="ps_kv", bufs=1)
                        mm = nc.tensor.matmul(psk[:], bdiag_t[:, h, :], kv_t[:, h, :],
                                         start=True, stop=False)
                        _mark(f"t{t}.b{b}.h{h}.psk", mm)
                        nc.tensor.matmul(psk[:], kn[0][:], v_t[:, 2 * b, hsl],
                                         start=False, stop=False)
                        nc.tensor.matmul(psk[:], kn[1][:], v_t[:, 2 * b + 1, hsl],
                                         start=False, stop=True)
                        nc.vector.tensor_copy(kv_t[:, h, :], psk[:])

                    if t == NT - 1:
                        emit_tail(t, attn_t, gTt, b)
                    else:
                        pending.append((t, attn_t, gTt, b))


        if timing:
            prb = spool.tile([P, 4], FP32, name="prb")
            nc.sync.dma_start(prb[:], pout_d[0:P, 0:4])
            nc.sync.dma_start(probe_d[:], prb[:])

    nc.compile()
    return nc


_NC_CACHE = {}


def _get_nc(repeat=1, timing=False):
    key = (repeat, timing)
    if key not in _NC_CACHE:
        _NC_CACHE[key] = build_nc(repeat, timing)
    return _NC_CACHE[key]


def make_timing_in_maps():
    ones = np.ones((P, 1), dtype=BF)
    return [{"onesb": ones} for _ in range(N_CORES)]


def make_in_maps(inputs):
    hs = np.ascontiguousarray(np.asarray(inputs["hidden_states"], dtype=np.float32))
    w_qkv = np.asarray(inputs["w_qkv"], dtype=np.float32)
    w_gate = np.asarray(inputs["w_gate"], dtype=np.float32)
    w_out = np.asarray(inputs["w_out"], dtype=np.float32)
    norm_weight = np.asarray(inputs["norm_weight"], dtype=np.float32)
    slope_rate = np.asarray(inputs["slope_rate"], dtype=np.float32).reshape(NUM_HEADS)
    kv_cache = np.asarray(inputs["kv_cache"], dtype=np.float32)

    xtb = np.ascontiguousarray(hs.T).astype(BF)            # [HIDDEN, SEQ]
    wq3 = w_qkv.reshape(HIDDEN, NUM_HEADS, 3 * HEAD_DIM)
    # fold 0.5 (from sigmoid = 0.5*(1+tanh)) and norm_weight into w_out rows
    wo_scaled = w_out * (0.5 * norm_weight)[:, None]
    identb = np.eye(P, dtype=BF)
    onesb = np.ones((P, 1), dtype=BF)
    idx = np.arange(BLOCK, dtype=np.float64)

    in_maps = []
    for c in range(N_CORES):
        s = slope_rate[c * HPC:(c + 1) * HPC].astype(np.float64)  # [HPC]
        wq = np.ascontiguousarray(
            wq3[:, c * HPC:(c + 1) * HPC, 0:HEAD_DIM].reshape(HIDDEN, IN_PC))
        wk = np.ascontiguousarray(
            wq3[:, c * HPC:(c + 1) * HPC, HEAD_DIM:2 * HEAD_DIM].reshape(HIDDEN, IN_PC))
        wv = np.ascontiguousarray(
            wq3[:, c * HPC:(c + 1) * HPC, 2 * HEAD_DIM:3 * HEAD_DIM].reshape(HIDDEN, IN_PC))
        wg = np.ascontiguousarray(w_gate[:, c * IN_PC:(c + 1) * IN_PC])
        wo = np.ascontiguousarray(wo_scaled[c * IN_PC:(c + 1) * IN_PC, :])

        jj = idx[:128][:, None]                          # [128,1]
        ii = idx[None, :]                                # [1,256]
        d0 = np.exp(-s[:, None, None] * (ii - jj)) * (ii >= jj)
        dmask0 = d0.astype(np.float32)                   # [HPC,128,256]
        dmask1 = np.zeros((HPC, P, BLOCK), dtype=np.float32)
        dmask1[:, :, P:] = dmask0[:, :, :P]
        qdec = np.broadcast_to(
            np.exp(-s[:, None] * (idx[None, :] + 1.0))[:, None, :],
            (HPC, P, BLOCK)).astype(BF)
        kdec = np.exp(-s[:, None] * (BLOCK - 1.0 - idx[None, :]))  # [HPC, 256]
        kdec = kdec.reshape(HPC, 2, P, 1).astype(np.float32)
        bd = np.exp(-s * BLOCK)                          # [HPC]
        bdiag = (np.eye(P, dtype=np.float64)[:, None, :]
                 * bd[None, :, None]).astype(BF)         # [P, HPC, P]
        kv0 = np.ascontiguousarray(kv_cache[c * HPC:(c + 1) * HPC]).astype(BF)

        in_maps.append({
            "xtb": xtb, "wqb": wq.astype(BF), "wkb": wk.astype(BF),
            "wvb": wv.astype(BF), "wgb": wg.astype(BF), "wob": wo.astype(BF),
            "qdec": np.ascontiguousarray(qdec), "dmask0": dmask0,
            "dmask1": dmask1, "kdec": kdec, "bdiag": np.ascontiguousarray(bdiag),
            "identb": identb, "onesb": onesb, "kv0b": kv0,
        })
    return in_maps


def combine_outputs(results):
    pout = np.zeros((SEQ, HIDDEN), dtype=np.float64)
    ssq = np.zeros((SEQ,), dtype=np.float64)
    for r in results:
        pout += r["pout"].astype(np.float64)
        ssq += r["ssq"].reshape(SEQ).astype(np.float64)
    var = ssq / INNER
    scale = 1.0 / np.sqrt(var + EPS)
    return (pout * scale[:, None]).astype(np.float32)


def kernel(**inputs):
    nc = _get_nc(1)
    in_maps = make_in_maps(inputs)
    last_err = None
    for _attempt in range(3):
        try:
            res = run_bass_kernel_spmd(nc, in_maps, core_ids=list(range(N_CORES)))
            return combine_outputs(res.results)
        except Exception as e:  # transient NRT_EXEC_UNIT_UNRECOVERABLE seen on axon
            last_err = e
    raise last_err


# revision 4
# speedup vs baseline: 533.0940x; 533.0940x over previous
"""MiniMaxText01 linear attention on 8 trn2 NeuronCores — fused bf16 version.

Tensor-parallel over heads (4 heads/core). Single fused pass per core:
for each 512-token chunk: qkv/gate projections (bf16 matmuls, weights
SBUF-resident), blocked lightning attention (BLOCK=256, kv state in SBUF
bf16, per-block decay folded into a diagonal-matrix matmul), gating
(sigmoid via tanh so the ACT engine never switches table sets), and the
out projection. Each core emits:
  pout = (gate * attn * 0.5*norm_w) @ w_out   [4096, 2048]  (0.5*normw
         pre-folded into w_out rows on the host)
  ssq  = sum over this core's channels of attn^2   [1, 4096]
Host: out = sum_c(pout) * rsqrt(sum_c(ssq)/4096 + eps).
"""
import math
import numpy as np
from contextlib import ExitStack

import ml_dtypes
import concourse.bass as bass
import concourse.tile as tile
import concourse.mybir as mybir
from concourse import bacc
from concourse.bass_utils import run_bass_kernel_spmd

FP32 = mybir.dt.float32
BF16 = mybir.dt.bfloat16
AF = mybir.ActivationFunctionType
BF = ml_dtypes.bfloat16

SEQ = 4096
HIDDEN = 2048
NUM_HEADS = 32
HEAD_DIM = 128
INNER = NUM_HEADS * HEAD_DIM
BLOCK = 256
EPS = 1e-5
N_CORES = 8
HPC = NUM_HEADS // N_CORES          # 4 heads per core
IN_PC = HPC * HEAD_DIM              # 512 inner channels per core
P = 128

CH = 512                            # tokens per chunk
NT = SEQ // CH                      # 8 chunks
HC = HIDDEN // P                    # 16 hidden sub-blocks


REGION_LOG = []


def _mark(label, ins):
    try:
        REGION_LOG.append((label, ins.ins.name))
    except Exception:
        pass


def build_nc(repeat: int = 1, timing: bool = False):
    REGION_LOG.clear()
    nc = bacc.Bacc("TRN2", target_bir_lowering=False)

    # In timing mode every big tensor is Internal (device-resident garbage)
    # so per-call host<->device traffic is tiny and the R-repeat wall-clock
    # delta isolates pure HW execution time.
    big = {"kind": "Internal"} if timing else {"kind": "ExternalInput"}
    xt_d = nc.dram_tensor("xtb", [HIDDEN, SEQ], BF16, **big)
    wq_d = nc.dram_tensor("wqb", [HIDDEN, IN_PC], BF16, **big)
    wk_d = nc.dram_tensor("wkb", [HIDDEN, IN_PC], BF16, **big)
    wv_d = nc.dram_tensor("wvb", [HIDDEN, IN_PC], BF16, **big)
    wg_d = nc.dram_tensor("wgb", [HIDDEN, IN_PC], BF16, **big)
    wo_d = nc.dram_tensor("wob", [IN_PC, HIDDEN], BF16, **big)
    qdec_d = nc.dram_tensor("qdec", [HPC, P, BLOCK], BF16, **big)
    dmask0_d = nc.dram_tensor("dmask0", [HPC, P, BLOCK], FP32, **big)
    dmask1_d = nc.dram_tensor("dmask1", [HPC, P, BLOCK], FP32, **big)
    kdec_d = nc.dram_tensor("kdec", [HPC, 2, P, 1], FP32, **big)
    bdiag_d = nc.dram_tensor("bdiag", [P, HPC, P], BF16, **big)
    ident_d = nc.dram_tensor("identb", [P, P], BF16, **big)
    ones_d = nc.dram_tensor("onesb", [P, 1], BF16, kind="ExternalInput")
    kv0_d = nc.dram_tensor("kv0b", [HPC, P, P], BF16, **big)

    if timing:
        pout_d = nc.dram_tensor("pout", [SEQ, HIDDEN], FP32, kind="Internal")
        probe_d = nc.dram_tensor("probe", [P, 4], FP32, kind="ExternalOutput")
    else:
        pout_d = nc.dram_tensor("pout", [SEQ, HIDDEN], FP32, kind="ExternalOutput")
    ssq_d = nc.dram_tensor("ssq", [1, SEQ], FP32, kind="ExternalOutput")

    with tile.TileContext(nc) as tc, ExitStack() as ctx:
        const = ctx.enter_context(tc.tile_pool(name="const", bufs=1))
        wpool = ctx.enter_context(tc.tile_pool(name="w", bufs=1))
        xpool = ctx.enter_context(tc.tile_pool(name="x", bufs=1))
        apool = ctx.enter_context(tc.tile_pool(name="a", bufs=1))
        spool = ctx.enter_context(tc.tile_pool(name="s", bufs=1))
        psum = ctx.enter_context(tc.tile_pool(name="psum", bufs=1, space="PSUM"))

        HH = HC // 2
        xt_r = xt_d[:].rearrange("(hc p) n -> p hc n", p=P)

        # startup order: wq/xh0 interleaved so the first matmul chain can
        # begin ~6us in; wo (needed last) loads last.
        wq_t = wpool.tile([P, HC, IN_PC], BF16)
        wk_t = wpool.tile([P, HC, IN_PC], BF16)
        wv_t = wpool.tile([P, HC, IN_PC], BF16)
        wg_t = wpool.tile([P, HC, IN_PC], BF16)
        wo_t = wpool.tile([P, HPC, HIDDEN], BF16)
        wq_r = wq_d[:].rearrange("(hc p) m -> p hc m", p=P)

        xh0 = xpool.tile([P, HC, CH], BF16, tag="xh", bufs=2)
        for half in range(2):
            nc.sync.dma_start(wq_t[:, half * HH:(half + 1) * HH, :],
                              wq_r[:, half * HH:(half + 1) * HH, :])
            nc.sync.dma_start(xh0[:, half * HH:(half + 1) * HH, :],
                              xt_r[:, half * HH:(half + 1) * HH, 0:CH])
        for w_t, w_d in ((wk_t, wk_d), (wv_t, wv_d), (wg_t, wg_d)):
            w_r = w_d[:].rearrange("(hc p) m -> p hc m", p=P)
            for half in range(2):
                nc.sync.dma_start(w_t[:, half * HH:(half + 1) * HH, :],
                                  w_r[:, half * HH:(half + 1) * HH, :])

        ident_t = const.tile([P, P], BF16)
        nc.sync.dma_start(ident_t[:], ident_d[:])
        qdec_t = const.tile([P, HPC, BLOCK], BF16)
        nc.sync.dma_start(qdec_t[:], qdec_d[:].rearrange("h p i -> p h i"))
        dmask0_t = const.tile([P, HPC, BLOCK], FP32)
        nc.sync.dma_start(dmask0_t[:], dmask0_d[:].rearrange("h p i -> p h i"))
        dmask1_t = const.tile([P, HPC, BLOCK], FP32)
        nc.sync.dma_start(dmask1_t[:], dmask1_d[:].rearrange("h p i -> p h i"))
        kdec_t = const.tile([P, HPC, 2, 1], FP32)
        nc.sync.dma_start(kdec_t[:], kdec_d[:].rearrange("h s p o -> p h s o"))
        bdiag_t = const.tile([P, HPC, P], BF16)
        nc.sync.dma_start(bdiag_t[:], bdiag_d[:])
        ones_t = const.tile([P, 1], BF16)
        nc.sync.dma_start(ones_t[:], ones_d[:])
        nc.sync.dma_start(wo_t[:], wo_d[:].rearrange("(h p) n -> p h n", p=P))
        kv_t = const.tile([P, HPC, P], BF16)

        for _rep in range(repeat):
            nc.sync.dma_start(kv_t[:], kv0_d[:].rearrange("h d e -> d h e"))

            pending = []          # (t, attn_t, gTt, b) awaiting gating/ssq/outproj
            tail_state = {}       # t -> (gA, sq) tiles

            def emit_tail(t, attn_t, gTt, b):
                # gating for half-chunk b (0.5*normw folded into w_out):
                # gA = attn*(1+tanh)
                if t not in tail_state:
                    gA = apool.tile([P, HPC, CH], BF16, tag="gA", bufs=2, name=f"gA{t}")
                    sq = apool.tile([P, HPC, CH], BF16, tag="sq", bufs=2, name=f"sq{t}")
                    tail_state[t] = (gA, sq)
                gA, sq = tail_state[t]
                bsl = slice(b * BLOCK, (b + 1) * BLOCK)
                nc.vector.tensor_mul(gA[:, :, bsl], attn_t[:, :, bsl], gTt[:, :, bsl])
                nc.vector.tensor_add(gA[:, :, bsl], gA[:, :, bsl], attn_t[:, :, bsl])
                nc.vector.tensor_mul(sq[:, :, bsl], attn_t[:, :, bsl], attn_t[:, :, bsl])

                # ssq = ones^T @ sq for this half
                pss = psum.tile([1, BLOCK], FP32, tag="psA", bufs=2)
                for h in range(HPC):
                    mm = nc.tensor.matmul(pss[:], ones_t[:], sq[:, h, bsl],
                                     start=(h == 0), stop=(h == HPC - 1))
                    if h == 0:
                        _mark(f"t{t}.ssq", mm)
                ssb = spool.tile([1, BLOCK], FP32, tag="ssb", bufs=2)
                nc.scalar.copy(ssb[:], pss[:])
                nc.sync.dma_start(
                    ssq_d[:, t * CH + b * BLOCK: t * CH + (b + 1) * BLOCK], ssb[:])

                # out projection for the two m-tiles of this half
                for m in (2 * b, 2 * b + 1):
                    ob = spool.tile([P, HIDDEN], FP32, tag="ob", bufs=2)
                    for nt in range(4):
                        ps = psum.tile([P, 512], FP32, tag="psA", bufs=2)
                        for h in range(HPC):
                            mm = nc.tensor.matmul(
                                ps[:], gA[:, h, m * P:(m + 1) * P],
                                wo_t[:, h, nt * 512:(nt + 1) * 512],
                                start=(h == 0), stop=(h == HPC - 1))
                            if h == 0:
                                _mark(f"t{t}.op.m{m}.n{nt}", mm)
                        nc.vector.tensor_copy(ob[:, nt * 512:(nt + 1) * 512], ps[:])
                        if nt == 1:
                            nc.sync.dma_start(
                                pout_d[t * CH + m * P: t * CH + (m + 1) * P, 0:1024],
                                ob[:, 0:1024])
                    nc.sync.dma_start(
                        pout_d[t * CH + m * P: t * CH + (m + 1) * P, 1024:HIDDEN],
                        ob[:, 1024:HIDDEN])

            for t in range(NT):
                tsl = slice(t * CH, (t + 1) * CH)
                if t == 0:
                    xh = xh0
                else:
                    xh = xpool.tile([P, HC, CH], BF16, tag="xh", bufs=2)
                    for half in range(2):
                        nc.sync.dma_start(
                            xh[:, half * HH:(half + 1) * HH, :],
                            xt_r[:, half * HH:(half + 1) * HH, tsl])

                qT = apool.tile([P, HPC, CH], BF16, tag="qT", bufs=2)
                kT = apool.tile([P, HPC, CH], BF16, tag="kT", bufs=2)
                v_t = apool.tile([P, 4, IN_PC], BF16, tag="v", bufs=2)
                gTt = apool.tile([P, HPC, CH], BF16, tag="g", bufs=2)

                # q/k projections: psum chains over hidden, silu -> bf16
                for wi, (w_t, dst) in enumerate(((wq_t, qT), (wk_t, kT))):
                    for h in range(HPC):
                        ps = psum.tile([P, CH], FP32, tag="psA", bufs=2)
                        for hc in range(HC):
                            mm = nc.tensor.matmul(
                                ps[:], w_t[:, hc, h * P:(h + 1) * P],
                                xh[:, hc, :],
                                start=(hc == 0), stop=(hc == HC - 1))
                            if hc == 0:
                                _mark(f"t{t}.qk{wi}.h{h}.mm", mm)
                        nc.scalar.activation(dst[:, h, :], ps[:], AF.Silu)

                # v projection (natural layout), silu -> bf16
                for t2 in range(4):
                    ps = psum.tile([P, IN_PC], FP32, tag="psA", bufs=2)
                    for hc in range(HC):
                        mm = nc.tensor.matmul(
                            ps[:], xh[:, hc, t2 * P:(t2 + 1) * P],
                            wv_t[:, hc, :],
                            start=(hc == 0), stop=(hc == HC - 1))
                        if hc == 0:
                            _mark(f"t{t}.v.{t2}.mm", mm)
                    nc.scalar.activation(v_t[:, t2, :], ps[:], AF.Silu)

                # gate projection: tanh(x/2) -> bf16 (gate = 0.5*(1+tanh))
                for h in range(HPC):
                    ps = psum.tile([P, CH], FP32, tag="psA", bufs=2)
                    for hc in range(HC):
                        mm = nc.tensor.matmul(
                            ps[:], wg_t[:, hc, h * P:(h + 1) * P],
                            xh[:, hc, :],
                            start=(hc == 0), stop=(hc == HC - 1))
                        if hc == 0:
                            _mark(f"t{t}.g.h{h}.mm", mm)
                    nc.scalar.activation(gTt[:, h, :], ps[:], AF.Tanh, scale=0.5)

                # pipeline: previous chunk's gating/ssq/outproj emitted here so
                # its PE work interleaves with this chunk's attention stalls
                while pending:
                    emit_tail(*pending.pop(0))
                tail_state.pop(t - 2, None)

                attn_t = apool.tile([P, HPC, CH], BF16, tag="attn", bufs=2)

                for b in range(CH // BLOCK):
                    t0 = b * BLOCK
                    for h in range(HPC):
                        hsl = slice(h * P, (h + 1) * P)
                        # scores (transposed): sT[j, i] = k_j . q_i
                        ps01 = psum.tile([P, 2, BLOCK], FP32, tag="ps_s", bufs=1)
                        mm = nc.tensor.matmul(ps01[:, 0, :], kT[:, h, t0:t0 + P],
                                         qT[:, h, t0:t0 + BLOCK],
                                         start=True, stop=True)
                        _mark(f"t{t}.b{b}.h{h}.score", mm)
                        nc.tensor.matmul(ps01[:, 1, :], kT[:, h, t0 + P:t0 + BLOCK],
                                         qT[:, h, t0:t0 + BLOCK],
                                         start=True, stop=True)
                        s0 = spool.tile([P, BLOCK], BF16, tag="s0", bufs=4)
                        nc.vector.tensor_mul(s0[:], ps01[:, 0, :], dmask0_t[:, h, :])
                        s1 = spool.tile([P, BLOCK], BF16, tag="s1", bufs=4)
                        nc.vector.tensor_mul(s1[:], ps01[:, 1, :], dmask1_t[:, h, :])
                        qd = spool.tile([P, BLOCK], BF16, tag="qd", bufs=4)
                        nc.vector.tensor_mul(qd[:], qT[:, h, t0:t0 + BLOCK],
                                             qdec_t[:, h, :])
                        # k natural (transposed back) with k-decay folded in
                        kn = []
                        pst = psum.tile([P, 2, P], BF16, tag="ps_tr", bufs=2)
                        for sub in range(2):
                            mm = nc.tensor.transpose(
                                pst[:, sub, :], kT[:, h, t0 + sub * P:t0 + (sub + 1) * P],
                                ident_t[:])
                            _mark(f"t{t}.b{b}.h{h}.tr{sub}", mm)
                            knt = spool.tile([P, P], BF16, tag=f"kn{sub}", bufs=4)
                            nc.scalar.activation(knt[:], pst[:, sub, :], AF.Copy,
                                                 scale=kdec_t[:, h, sub, :])
                            kn.append(knt)
                        # attention output (transposed): inter + intra
                        pso = psum.tile([P, BLOCK], FP32, tag="ps_o", bufs=2)
                        mm = nc.tensor.matmul(pso[:], kv_t[:, h, :], qd[:],
                                         start=True, stop=False)
                        _mark(f"t{t}.b{b}.h{h}.pso", mm)
                        nc.tensor.matmul(pso[:], v_t[:, 2 * b, hsl], s0[:],
                                         start=False, stop=False)
                        nc.tensor.matmul(pso[:], v_t[:, 2 * b + 1, hsl], s1[:],
                                         start=False, stop=True)
                        nc.scalar.copy(attn_t[:, h, t0:t0 + BLOCK], pso[:])
                        # kv update in psum: psk = bd*kv + (k*kdec)^T v
                        psk = psum.tile([P, P], FP32, tag="ps_kv", bufs=1)
                        mm = nc.tensor.matmul(psk[:], bdiag_t[:, h, :], kv_t[:, h, :],
                                         start=True, stop=False)
                        _mark(f"t{t}.b{b}.h{h}.psk", mm)
                        nc.tensor.matmul(psk[:], kn[0][:], v_t[:, 2 * b, hsl],
                                         start=False, stop=False)
                        nc.tensor.matmul(psk[:], kn[1][:], v_t[:, 2 * b + 1, hsl],
                                         start=False, stop=True)
                        nc.scalar.copy(kv_t[:, h, :], psk[:])

                    if t == NT - 1:
                        emit_tail(t, attn_t, gTt, b)
                    else:
                        pending.append((t, attn_t, gTt, b))


        if timing:
            prb = spool.tile([P, 4], FP32, name="prb")
            nc.sync.dma_start(prb[:], pout_d[0:P, 0:4])
            nc.sync.dma_start(probe_d[:], prb[:])

    nc.compile()
    return nc


_NC_CACHE = {}


def _get_nc(repeat=1, timing=False):
    key = (repeat, timing)
    if key not in _NC_CACHE:
        _NC_CACHE[key] = build_nc(repeat, timing)
    return _NC_CACHE[key]


def make_timing_in_maps():
    ones = np.ones((P, 1), dtype=BF)
    return [{"onesb": ones} for _ in range(N_CORES)]


def make_in_maps(inputs):
    hs = np.ascontiguousarray(np.asarray(inputs["hidden_states"], dtype=np.float32))
    w_qkv = np.asarray(inputs["w_qkv"], dtype=np.float32)
    w_gate = np.asarray(inputs["w_gate"], dtype=np.float32)
    w_out = np.asarray(inputs["w_out"], dtype=np.float32)
    norm_weight = np.asarray(inputs["norm_weight"], dtype=np.float32)
    slope_rate = np.asarray(inputs["slope_rate"], dtype=np.float32).reshape(NUM_HEADS)
    kv_cache = np.asarray(inputs["kv_cache"], dtype=np.float32)

    xtb = np.ascontiguousarray(hs.T).astype(BF)            # [HIDDEN, SEQ]
    wq3 = w_qkv.reshape(HIDDEN, NUM_HEADS, 3 * HEAD_DIM)
    # fold 0.5 (from sigmoid = 0.5*(1+tanh)) and norm_weight into w_out rows
    wo_scaled = w_out * (0.5 * norm_weight)[:, None]
    identb = np.eye(P, dtype=BF)
    onesb = np.ones((P, 1), dtype=BF)
    idx = np.arange(BLOCK, dtype=np.float64)

    in_maps = []
    for c in range(N_CORES):
        s = slope_rate[c * HPC:(c + 1) * HPC].astype(np.float64)  # [HPC]
        wq = np.ascontiguousarray(
            wq3[:, c * HPC:(c + 1) * HPC, 0:HEAD_DIM].reshape(HIDDEN, IN_PC))
        wk = np.ascontiguousarray(
            wq3[:, c * HPC:(c + 1) * HPC, HEAD_DIM:2 * HEAD_DIM].reshape(HIDDEN, IN_PC))
        wv = np.ascontiguousarray(
            wq3[:, c * HPC:(c + 1) * HPC, 2 * HEAD_DIM:3 * HEAD_DIM].reshape(HIDDEN, IN_PC))
        wg = np.ascontiguousarray(w_gate[:, c * IN_PC:(c + 1) * IN_PC])
        wo = np.ascontiguousarray(wo_scaled[c * IN_PC:(c + 1) * IN_PC, :])

        jj = idx[:128][:, None]                          # [128,1]
        ii = idx[None, :]                                # [1,256]
        d0 = np.exp(-s[:, None, None] * (ii - jj)) * (ii >= jj)
        dmask0 = d0.astype(np.float32)                   # [HPC,128,256]
        dmask1 = np.zeros((HPC, P, BLOCK), dtype=np.float32)
        dmask1[:, :, P:] = dmask0[:, :, :P]
        qdec = np.broadcast_to(
            np.exp(-s[:, None] * (idx[None, :] + 1.0))[:, None, :],
            (HPC, P, BLOCK)).astype(BF)
        kdec = np.exp(-s[:, None] * (BLOCK - 1.0 - idx[None, :]))  # [HPC, 256]
        kdec = kdec.reshape(HPC, 2, P, 1).astype(np.float32)
        bd = np.exp(-s * BLOCK)                          # [HPC]
        bdiag = (np.eye(P, dtype=np.float64)[:, None, :]
                 * bd[None, :, None]).astype(BF)         # [P, HPC, P]
        kv0 = np.ascontiguousarray(kv_cache[c * HPC:(c + 1) * HPC]).astype(BF)

        in_maps.append({
            "xtb": xtb, "wqb": wq.astype(BF), "wkb": wk.astype(BF),
            "wvb": wv.astype(BF), "wgb": wg.astype(BF), "wob": wo.astype(BF),
            "qdec": np.ascontiguousarray(qdec), "dmask0": dmask0,
            "dmask1": dmask1, "kdec": kdec, "bdiag": np.ascontiguousarray(bdiag),
            "identb": identb, "onesb": onesb, "kv0b": kv0,
        })
    return in_maps


def combine_outputs(results):
    pout = np.zeros((SEQ, HIDDEN), dtype=np.float64)
    ssq = np.zeros((SEQ,), dtype=np.float64)
    for r in results:
        pout += r["pout"].astype(np.float64)
        ssq += r["ssq"].reshape(SEQ).astype(np.float64)
    var = ssq / INNER
    scale = 1.0 / np.sqrt(var + EPS)
    return (pout * scale[:, None]).astype(np.float32)


def kernel(**inputs):
    nc = _get_nc(1)
    in_maps = make_in_maps(inputs)
    last_err = None
    for _attempt in range(3):
        try:
            res = run_bass_kernel_spmd(nc, in_maps, core_ids=list(range(N_CORES)))
            return combine_outputs(res.results)
        except Exception as e:  # transient NRT_EXEC_UNIT_UNRECOVERABLE seen on axon
            last_err = e
    raise last_err


# revision 7
# speedup vs baseline: 543.4309x; 1.0194x over previous
"""MiniMaxText01 linear attention on 8 trn2 NeuronCores — fused bf16 version.

Tensor-parallel over heads (4 heads/core). Single fused pass per core:
for each 512-token chunk: qkv/gate projections (bf16 matmuls, weights
SBUF-resident), blocked lightning attention (BLOCK=256, kv state in SBUF
bf16, per-block decay folded into a diagonal-matrix matmul), gating
(sigmoid via tanh so the ACT engine never switches table sets), and the
out projection. Each core emits:
  pout = (gate * attn * 0.5*norm_w) @ w_out   [4096, 2048]  (0.5*normw
         pre-folded into w_out rows on the host)
  ssq  = sum over this core's channels of attn^2   [1, 4096]
Host: out = sum_c(pout) * rsqrt(sum_c(ssq)/4096 + eps).
"""
import math
import numpy as np
from contextlib import ExitStack

import ml_dtypes
import concourse.bass as bass
import concourse.bass_isa as bass_isa
import concourse.tile as tile
import concourse.mybir as mybir
from concourse import bacc
from concourse.bass_utils import run_bass_kernel_spmd

FP32 = mybir.dt.float32
BF16 = mybir.dt.bfloat16
AF = mybir.ActivationFunctionType
BF = ml_dtypes.bfloat16

SEQ = 4096
HIDDEN = 2048
NUM_HEADS = 32
HEAD_DIM = 128
INNER = NUM_HEADS * HEAD_DIM
BLOCK = 256
EPS = 1e-5
N_CORES = 8
HPC = NUM_HEADS // N_CORES          # 4 heads per core
IN_PC = HPC * HEAD_DIM              # 512 inner channels per core
P = 128

CH = 512                            # tokens per chunk
NT = SEQ // CH                      # 8 chunks
HC = HIDDEN // P                    # 16 hidden sub-blocks


REGION_LOG = []


def _mark(label, ins):
    try:
        REGION_LOG.append((label, ins.ins.name))
    except Exception:
        pass


def build_nc(repeat: int = 1, timing: bool = False):
    REGION_LOG.clear()
    nc = bacc.Bacc("TRN2", target_bir_lowering=False)

    # In timing mode every big tensor is Internal (device-resident garbage)
    # so per-call host<->device traffic is tiny and the R-repeat wall-clock
    # delta isolates pure HW execution time.
    big = {"kind": "Internal"} if timing else {"kind": "ExternalInput"}
    xt_d = nc.dram_tensor("xtb", [HIDDEN, SEQ], BF16, **big)
    wq_d = nc.dram_tensor("wqb", [HIDDEN, IN_PC], BF16, **big)
    wk_d = nc.dram_tensor("wkb", [HIDDEN, IN_PC], BF16, **big)
    wv_d = nc.dram_tensor("wvb", [HIDDEN, IN_PC], BF16, **big)
    wg_d = nc.dram_tensor("wgb", [HIDDEN, IN_PC], BF16, **big)
    wo_d = nc.dram_tensor("wob", [IN_PC, HIDDEN], BF16, **big)
    qdec_d = nc.dram_tensor("qdec", [HPC, P, BLOCK], BF16, **big)
    dmask0_d = nc.dram_tensor("dmask0", [HPC, P, BLOCK], FP32, **big)
    dmask1_d = nc.dram_tensor("dmask1", [HPC, P, BLOCK], FP32, **big)
    kdec_d = nc.dram_tensor("kdec", [HPC, 2, P, 1], FP32, **big)
    bdiag_d = nc.dram_tensor("bdiag", [P, HPC, P], BF16, **big)
    ident_d = nc.dram_tensor("identb", [P, P], BF16, **big)
    ones_d = nc.dram_tensor("onesb", [P, 1], BF16, kind="ExternalInput")
    kv0_d = nc.dram_tensor("kv0b", [HPC, P, P], BF16, **big)

    if timing:
        pout_d = nc.dram_tensor("pout", [SEQ, HIDDEN], BF16, kind="Internal")
        probe_d = nc.dram_tensor("probe", [P, 4], BF16, kind="ExternalOutput")
    else:
        pout_d = nc.dram_tensor("pout", [SEQ, HIDDEN], BF16, kind="ExternalOutput")
    ssq_d = nc.dram_tensor("ssq", [1, SEQ], FP32, kind="ExternalOutput")

    with tile.TileContext(nc) as tc, ExitStack() as ctx:
        const = ctx.enter_context(tc.tile_pool(name="const", bufs=1))
        wpool = ctx.enter_context(tc.tile_pool(name="w", bufs=1))
        xpool = ctx.enter_context(tc.tile_pool(name="x", bufs=1))
        apool = ctx.enter_context(tc.tile_pool(name="a", bufs=1))
        spool = ctx.enter_context(tc.tile_pool(name="s", bufs=1))
        psum = ctx.enter_context(tc.tile_pool(name="psum", bufs=1, space="PSUM"))

        HH = HC // 2
        xt_r = xt_d[:].rearrange("(hc p) n -> p hc n", p=P)

        # startup order: wq/xh0 interleaved so the first matmul chain can
        # begin ~6us in; wo (needed last) loads last.
        wq_t = wpool.tile([P, HC, IN_PC], BF16)
        wk_t = wpool.tile([P, HC, IN_PC], BF16)
        wv_t = wpool.tile([P, HC, IN_PC], BF16)
        wg_t = wpool.tile([P, HC, IN_PC], BF16)
        wo_t = wpool.tile([P, HPC, HIDDEN], BF16)
        wq_r = wq_d[:].rearrange("(hc p) m -> p hc m", p=P)

        # chunk schedule: short first/last chunks shrink the DMA-bound
        # startup gap and the exposed tail (a 256-token chunk is exactly one
        # attention block, so all decay masks are unchanged)
        CHUNKS = [(0, 256)] + [(256 + 512 * i, 512) for i in range(7)] + [(3840, 256)]

        xh0 = xpool.tile([P, HC, 256], BF16, tag="xh", bufs=2)
        HQ0 = HC // 4
        for q in range(4):
            nc.sync.dma_start(wq_t[:, q * HQ0:(q + 1) * HQ0, :],
                              wq_r[:, q * HQ0:(q + 1) * HQ0, :])
            nc.sync.dma_start(xh0[:, q * HQ0:(q + 1) * HQ0, :],
                              xt_r[:, q * HQ0:(q + 1) * HQ0, 0:256])
        for w_t, w_d in ((wk_t, wk_d), (wv_t, wv_d), (wg_t, wg_d)):
            w_r = w_d[:].rearrange("(hc p) m -> p hc m", p=P)
            for half in range(2):
                nc.sync.dma_start(w_t[:, half * HH:(half + 1) * HH, :],
                                  w_r[:, half * HH:(half + 1) * HH, :])

        ident_t = const.tile([P, P], BF16)
        nc.sync.dma_start(ident_t[:], ident_d[:])
        qdec_t = const.tile([P, HPC, BLOCK], BF16)
        nc.sync.dma_start(qdec_t[:], qdec_d[:].rearrange("h p i -> p h i"))
        dmask0_t = const.tile([P, HPC, BLOCK], FP32)
        nc.sync.dma_start(dmask0_t[:], dmask0_d[:].rearrange("h p i -> p h i"))
        dmask1_t = const.tile([P, HPC, BLOCK], FP32)
        nc.sync.dma_start(dmask1_t[:], dmask1_d[:].rearrange("h p i -> p h i"))
        kdec_t = const.tile([P, HPC, 2, 1], FP32)
        nc.sync.dma_start(kdec_t[:], kdec_d[:].rearrange("h s p o -> p h s o"))
        bdiag_t = const.tile([P, HPC, P], BF16)
        nc.sync.dma_start(bdiag_t[:], bdiag_d[:])
        ones_t = const.tile([P, 1], BF16)
        nc.sync.dma_start(ones_t[:], ones_d[:])
        nc.sync.dma_start(wo_t[:], wo_d[:].rearrange("(h p) n -> p h n", p=P))
        kv_t = const.tile([P, HPC, P], BF16)

        for _rep in range(repeat):
            nc.sync.dma_start(kv_t[:], kv0_d[:].rearrange("h d e -> d h e"))

            pending = []          # (t, attn_t, gTt, b) awaiting gating/ssq/outproj
            tail_state = {}       # t -> (gA, sq) tiles

            def emit_tail(t, start, attn_t, gTt, b, fine=False):
                # gating for block b of chunk t (0.5*normw folded into w_out):
                # gA = attn*(1+tanh); fine=True (last chunk) emits per-head so
                # the out-projection starts as soon as each head's attn lands
                if t not in tail_state:
                    gA = apool.tile([P, HPC, CH], BF16, tag="gA", bufs=2, name=f"gA{t}")
                    sq = apool.tile([P, HPC, CH], BF16, tag="sq", bufs=2, name=f"sq{t}")
                    tail_state[t] = (gA, sq)
                gA, sq = tail_state[t]
                bsl = slice(b * BLOCK, (b + 1) * BLOCK)
                hgroups = [slice(h, h + 1) for h in range(HPC)] if fine \
                    else [slice(0, HPC)]
                for hs in hgroups:
                    nc.vector.tensor_mul(gA[:, hs, bsl], attn_t[:, hs, bsl],
                                         gTt[:, hs, bsl])
                    nc.vector.tensor_add(gA[:, hs, bsl], gA[:, hs, bsl],
                                         attn_t[:, hs, bsl])
                    nc.vector.tensor_mul(sq[:, hs, bsl], attn_t[:, hs, bsl],
                                         attn_t[:, hs, bsl])

                # ssq = sum over channels of sq — cross-partition reduce on
                # the otherwise-idle GpSimd engine (frees PE matmuls)
                rh = []
                for h in range(HPC):
                    r = spool.tile([P, BLOCK], FP32, tag=f"rh{h % 2}", bufs=2,
                                   name=f"rh{t}_{b}_{h}")
                    nc.gpsimd.partition_all_reduce(r[:], sq[:, h, bsl], P,
                                                   bass_isa.ReduceOp.add)
                    rh.append(r)
                ssb = spool.tile([1, BLOCK], FP32, tag="ssb", bufs=2)
                nc.gpsimd.tensor_add(rh[0][0:1, :], rh[0][0:1, :], rh[1][0:1, :])
                nc.gpsimd.tensor_add(rh[2][0:1, :], rh[2][0:1, :], rh[3][0:1, :])
                nc.gpsimd.tensor_add(ssb[:], rh[0][0:1, :], rh[2][0:1, :])
                nc.sync.dma_start(
                    ssq_d[:, start + b * BLOCK: start + (b + 1) * BLOCK], ssb[:])

                # out projection for the two m-tiles of this block
                for m in (2 * b, 2 * b + 1):
                    ob = spool.tile([P, HIDDEN], BF16, tag="ob", bufs=2)
                    for nt in range(4):
                        ps = psum.tile([P, 512], FP32, tag="psA", bufs=2)
                        for h in range(HPC):
                            mm = nc.tensor.matmul(
                                ps[:], gA[:, h, m * P:(m + 1) * P],
                                wo_t[:, h, nt * 512:(nt + 1) * 512],
                                start=(h == 0), stop=(h == HPC - 1))
                            if h == 0:
                                _mark(f"t{t}.op.m{m}.n{nt}", mm)
                        nc.vector.tensor_copy(ob[:, nt * 512:(nt + 1) * 512], ps[:])
                        if fine:
                            nc.sync.dma_start(
                                pout_d[start + m * P: start + (m + 1) * P,
                                       nt * 512:(nt + 1) * 512],
                                ob[:, nt * 512:(nt + 1) * 512])
                        elif nt == 1:
                            nc.sync.dma_start(
                                pout_d[start + m * P: start + (m + 1) * P, 0:1024],
                                ob[:, 0:1024])
                    if not fine:
                        nc.sync.dma_start(
                            pout_d[start + m * P: start + (m + 1) * P, 1024:HIDDEN],
                            ob[:, 1024:HIDDEN])

            for t, (start, ch) in enumerate(CHUNKS):
                tsl = slice(start, start + ch)
                if t == 0:
                    xh = xh0
                else:
                    xh = xpool.tile([P, HC, CH], BF16, tag="xh", bufs=2,
                                    name=f"xh{t}")
                    HQ = HC // 4
                    for q in range(4):
                        nc.sync.dma_start(
                            xh[:, q * HQ:(q + 1) * HQ, 0:ch],
                            xt_r[:, q * HQ:(q + 1) * HQ, tsl])

                qT = apool.tile([P, HPC, CH], BF16, tag="qT", bufs=2)
                kT = apool.tile([P, HPC, CH], BF16, tag="kT", bufs=2)
                v_t = apool.tile([P, 4, IN_PC], BF16, tag="v", bufs=2)
                gTt = apool.tile([P, HPC, CH], BF16, tag="g", bufs=2)

                # q/k projections: psum chains over hidden, silu -> bf16
                for wi, (w_t, dst) in enumerate(((wq_t, qT), (wk_t, kT))):
                    for h in range(HPC):
                        ps = psum.tile([P, CH], FP32, tag="psA", bufs=2)
                        for hc in range(HC):
                            mm = nc.tensor.matmul(
                                ps[:, 0:ch], w_t[:, hc, h * P:(h + 1) * P],
                                xh[:, hc, 0:ch],
                                start=(hc == 0), stop=(hc == HC - 1))
                            if hc == 0:
                                _mark(f"t{t}.qk{wi}.h{h}.mm", mm)
                        nc.scalar.activation(dst[:, h, 0:ch], ps[:, 0:ch], AF.Silu)

                # v projection (natural layout), silu -> bf16
                for t2 in range(ch // P):
                    ps = psum.tile([P, IN_PC], FP32, tag="psA", bufs=2)
                    for hc in range(HC):
                        mm = nc.tensor.matmul(
                            ps[:], xh[:, hc, t2 * P:(t2 + 1) * P],
                            wv_t[:, hc, :],
                            start=(hc == 0), stop=(hc == HC - 1))
                        if hc == 0:
                            _mark(f"t{t}.v.{t2}.mm", mm)
                    nc.scalar.activation(v_t[:, t2, :], ps[:], AF.Silu)

                # gate projection: tanh(x/2) -> bf16 (gate = 0.5*(1+tanh))
                for h in range(HPC):
                    ps = psum.tile([P, CH], FP32, tag="psA", bufs=2)
                    for hc in range(HC):
                        mm = nc.tensor.matmul(
                            ps[:, 0:ch], wg_t[:, hc, h * P:(h + 1) * P],
                            xh[:, hc, 0:ch],
                            start=(hc == 0), stop=(hc == HC - 1))
                        if hc == 0:
                            _mark(f"t{t}.g.h{h}.mm", mm)
                    nc.scalar.activation(gTt[:, h, 0:ch], ps[:, 0:ch],
                                         AF.Tanh, scale=0.5)

                # pipeline: previous chunk's gating/ssq/outproj emitted here so
                # its PE work interleaves with this chunk's attention stalls
                while pending:
                    emit_tail(*pending.pop(0))
                tail_state.pop(t - 2, None)

                attn_t = apool.tile([P, HPC, CH], BF16, tag="attn", bufs=2)

                for b in range(ch // BLOCK):
                    t0 = b * BLOCK
                    for h in range(HPC):
                        hsl = slice(h * P, (h + 1) * P)
                        # scores (transposed): sT[j, i] = k_j . q_i
                        ps01 = psum.tile([P, 2, BLOCK], FP32, tag="ps_s", bufs=1)
                        mm = nc.tensor.matmul(ps01[:, 0, :], kT[:, h, t0:t0 + P],
                                         qT[:, h, t0:t0 + BLOCK],
                                         start=True, stop=True)
                        _mark(f"t{t}.b{b}.h{h}.score", mm)
                        nc.tensor.matmul(ps01[:, 1, :], kT[:, h, t0 + P:t0 + BLOCK],
                                         qT[:, h, t0:t0 + BLOCK],
                                         start=True, stop=True)
                        s0 = spool.tile([P, BLOCK], BF16, tag="s0", bufs=2)
                        nc.vector.tensor_mul(s0[:], ps01[:, 0, :], dmask0_t[:, h, :])
                        s1 = spool.tile([P, BLOCK], BF16, tag="s1", bufs=2)
                        nc.vector.tensor_mul(s1[:], ps01[:, 1, :], dmask1_t[:, h, :])
                        qd = spool.tile([P, BLOCK], BF16, tag="qd", bufs=2)
                        nc.vector.tensor_mul(qd[:], qT[:, h, t0:t0 + BLOCK],
                                             qdec_t[:, h, :])
                        # k natural (transposed back) with k-decay folded in
                        kn = []
                        pst = psum.tile([P, 2, P], BF16, tag="ps_tr", bufs=2)
                        for sub in range(2):
                            mm = nc.tensor.transpose(
                                pst[:, sub, :], kT[:, h, t0 + sub * P:t0 + (sub + 1) * P],
                                ident_t[:])
                            _mark(f"t{t}.b{b}.h{h}.tr{sub}", mm)
                            knt = spool.tile([P, P], BF16, tag=f"kn{sub}", bufs=2)
                            nc.scalar.activation(knt[:], pst[:, sub, :], AF.Copy,
                                                 scale=kdec_t[:, h, sub, :])
                            kn.append(knt)
                        # attention output (transposed): inter + intra
                        pso = psum.tile([P, BLOCK], FP32, tag="ps_o", bufs=2)
                        mm = nc.tensor.matmul(pso[:], kv_t[:, h, :], qd[:],
                                         start=True, stop=False)
                        _mark(f"t{t}.b{b}.h{h}.pso", mm)
                        nc.tensor.matmul(pso[:], v_t[:, 2 * b, hsl], s0[:],
                                         start=False, stop=False)
                        nc.tensor.matmul(pso[:], v_t[:, 2 * b + 1, hsl], s1[:],
                                         start=False, stop=True)
                        nc.scalar.copy(attn_t[:, h, t0:t0 + BLOCK], pso[:])
                        # kv update in psum: psk = bd*kv + (k*kdec)^T v
                        psk = psum.tile([P, P], FP32, tag="ps_kv", bufs=1)
                        mm = nc.tensor.matmul(psk[:], bdiag_t[:, h, :], kv_t[:, h, :],
                                         start=True, stop=False)
                        _mark(f"t{t}.b{b}.h{h}.psk", mm)
                        nc.tensor.matmul(psk[:], kn[0][:], v_t[:, 2 * b, hsl],
                                         start=False, stop=False)
                        nc.tensor.matmul(psk[:], kn[1][:], v_t[:, 2 * b + 1, hsl],
                                         start=False, stop=True)
                        nc.scalar.copy(kv_t[:, h, :], psk[:])

                    if t == len(CHUNKS) - 1:
                        emit_tail(t, start, attn_t, gTt, b, fine=True)
                    else:
                        pending.append((t, start, attn_t, gTt, b))


        if timing:
            prb = spool.tile([P, 4], BF16, name="prb")
            nc.sync.dma_start(prb[:], pout_d[0:P, 0:4])
            nc.sync.dma_start(probe_d[:], prb[:])

    nc.compile()
    return nc


_NC_CACHE = {}


def _get_nc(repeat=1, timing=False):
    key = (repeat, timing)
    if key not in _NC_CACHE:
        _NC_CACHE[key] = build_nc(repeat, timing)
    return _NC_CACHE[key]


def make_timing_in_maps():
    ones = np.ones((P, 1), dtype=BF)
    return [{"onesb": ones} for _ in range(N_CORES)]


def make_in_maps(inputs):
    hs = np.ascontiguousarray(np.asarray(inputs["hidden_states"], dtype=np.float32))
    w_qkv = np.asarray(inputs["w_qkv"], dtype=np.float32)
    w_gate = np.asarray(inputs["w_gate"], dtype=np.float32)
    w_out = np.asarray(inputs["w_out"], dtype=np.float32)
    norm_weight = np.asarray(inputs["norm_weight"], dtype=np.float32)
    slope_rate = np.asarray(inputs["slope_rate"], dtype=np.float32).reshape(NUM_HEADS)
    kv_cache = np.asarray(inputs["kv_cache"], dtype=np.float32)

    xtb = np.ascontiguousarray(hs.T).astype(BF)            # [HIDDEN, SEQ]
    wq3 = w_qkv.reshape(HIDDEN, NUM_HEADS, 3 * HEAD_DIM)
    # fold 0.5 (from sigmoid = 0.5*(1+tanh)) and norm_weight into w_out rows
    wo_scaled = w_out * (0.5 * norm_weight)[:, None]
    identb = np.eye(P, dtype=BF)
    onesb = np.ones((P, 1), dtype=BF)
    idx = np.arange(BLOCK, dtype=np.float64)

    in_maps = []
    for c in range(N_CORES):
        s = slope_rate[c * HPC:(c + 1) * HPC].astype(np.float64)  # [HPC]
        wq = np.ascontiguousarray(
            wq3[:, c * HPC:(c + 1) * HPC, 0:HEAD_DIM].reshape(HIDDEN, IN_PC))
        wk = np.ascontiguousarray(
            wq3[:, c * HPC:(c + 1) * HPC, HEAD_DIM:2 * HEAD_DIM].reshape(HIDDEN, IN_PC))
        wv = np.ascontiguousarray(
            wq3[:, c * HPC:(c + 1) * HPC, 2 * HEAD_DIM:3 * HEAD_DIM].reshape(HIDDEN, IN_PC))
        wg = np.ascontiguousarray(w_gate[:, c * IN_PC:(c + 1) * IN_PC])
        wo = np.ascontiguousarray(wo_scaled[c * IN_PC:(c + 1) * IN_PC, :])

        jj = idx[:128][:, None]                          # [128,1]
        ii = idx[None, :]                                # [1,256]
        d0 = np.exp(-s[:, None, None] * (ii - jj)) * (ii >= jj)
        dmask0 = d0.astype(np.float32)                   # [HPC,128,256]
        dmask1 = np.zeros((HPC, P, BLOCK), dtype=np.float32)
        dmask1[:, :, P:] = dmask0[:, :, :P]
        qdec = np.broadcast_to(
            np.exp(-s[:, None] * (idx[None, :] + 1.0))[:, None, :],
            (HPC, P, BLOCK)).astype(BF)
        kdec = np.exp(-s[:, None] * (BLOCK - 1.0 - idx[None, :]))  # [HPC, 256]
        kdec = kdec.reshape(HPC, 2, P, 1).astype(np.float32)
        bd = np.exp(-s * BLOCK)                          # [HPC]
        bdiag = (np.eye(P, dtype=np.float64)[:, None, :]
                 * bd[None, :, None]).astype(BF)         # [P, HPC, P]
        kv0 = np.ascontiguousarray(kv_cache[c * HPC:(c + 1) * HPC]).astype(BF)

        in_maps.append({
            "xtb": xtb, "wqb": wq.astype(BF), "wkb": wk.astype(BF),
            "wvb": wv.astype(BF), "wgb": wg.astype(BF), "wob": wo.astype(BF),
            "qdec": np.ascontiguousarray(qdec), "dmask0": dmask0,
            "dmask1": dmask1, "kdec": kdec, "bdiag": np.ascontiguousarray(bdiag),
            "identb": identb, "onesb": onesb, "kv0b": kv0,
        })
    return in_maps


def combine_outputs(results):
    pout = np.zeros((SEQ, HIDDEN), dtype=np.float64)
    ssq = np.zeros((SEQ,), dtype=np.float64)
    for r in results:
        pout += r["pout"].astype(np.float64)
        ssq += r["ssq"].reshape(SEQ).astype(np.float64)
    var = ssq / INNER
    scale = 1.0 / np.sqrt(var + EPS)
    return (pout * scale[:, None]).astype(np.float32)


def kernel(**inputs):
    nc = _get_nc(1)
    in_maps = make_in_maps(inputs)
    last_err = None
    for _attempt in range(3):
        try:
            res = run_bass_kernel_spmd(nc, in_maps, core_ids=list(range(N_CORES)))
            return combine_outputs(res.results)
        except Exception as e:  # transient NRT_EXEC_UNIT_UNRECOVERABLE seen on axon
            last_err = e
    raise last_err
